# revision 22
# baseline (speedup 1.0000x reference)
"""Trainium2 Bass kernel for multi-head self-attention with RoPE.

Sharding: 8 cores = 2 (batch) x 4 (head groups of 4 heads).
Each core computes its batch's attention for its 4 heads plus the
(row-sharded) output projection partial sum; the host adds the 4 head-group
partials per batch and the output bias.

Structure (v3):
- Q/K projections run in fp8(e4m3) DoubleRow mode (2x PE throughput); the
  4096x operand scaling is undone inside the exp() activation scale.
- V projection is t-outer over two 8-column-tile PSUM passes so it consumes
  the fp16 hidden tiles as their DMAs land.
- Attention per (chunk, head): score pairs -> one [128,1024] exp activation
  (scale fused) -> AV matmuls interleaved two pairs behind; the softmax
  denominator is pair-accumulated on vector, partition-reduced+broadcast by
  a single ones-matmul, inverted with reciprocal_approx_fast.
- Output projection: per-[128,512] PSUM groups, evacuation alternating
  scalar/vector, fp16 DMA alternating sync/gpsimd queues.
- Causal mask: one shared diagonal 512x512 block, multiplied into the two
  diagonal exp pairs; off-diagonal tiles are skipped (upper) or clean (lower).
"""

import sys
import types

import numpy as np

sys.path.insert(0, "/opt/trn_rl_repo")

# The axon boot registers its NTFF-profiling hook via antenv.axon_hooks; some
# images lack that module, which silently disables tracing. Provide it.
if "antenv.axon_hooks" not in sys.modules:
    try:
        import antenv.axon_hooks  # noqa: F401
    except ImportError:
        try:
            import antenv

            _m = types.ModuleType("antenv.axon_hooks")
            _m._hook = None
            _m.set_axon_ntff_profile_hook = lambda h: setattr(_m, "_hook", h)
            _m.get_axon_ntff_profile_hook = lambda: _m._hook
            sys.modules["antenv.axon_hooks"] = _m
            antenv.axon_hooks = _m
        except ImportError:
            pass

B, S, H, NH, HD = 2, 2048, 2048, 16, 128
ROPE_THETA = 10000.0
N_CORES = 8
HGRID = 4            # head-group shards
NHC = NH // HGRID    # heads per core

USE_FP8 = True       # fp8 e4m3 DoubleRow for the Q/K projections
FP8_HSCALE = 16.0    # hidden scale into fp8 (power of 2)
FP8_WSCALE = 256.0   # weight scale into fp8 (power of 2)

LAST_RESULTS = None  # test harness introspection
_CACHE = {}


def _rope_tables(S_, dtype=np.float16):
    # transposed rope tables [HD, S]; ss has rotate-half sign folded in:
    # rope(x)[d, s] = x[d, s]*cosT[d, s] + x[(d+64)%128, s]*ss[d, s]
    inv = 1.0 / (ROPE_THETA ** (np.arange(0, HD, 2, dtype=np.float64) / HD))
    t = np.arange(S_, dtype=np.float64)
    fr = np.outer(t, inv)                          # [S, HD/2]
    emb = np.concatenate([fr, fr], axis=1)         # [S, HD]
    cosT = np.cos(emb).T.astype(np.float32)        # [HD, S]
    ss = np.sin(emb).T.astype(np.float32)
    ss[: HD // 2] *= -1.0
    return cosT.astype(dtype), ss.astype(dtype)


def build_program(S_, H_, NHC_, use_fp8):
    """Build + compile the per-core SPMD bass program (causal mask layout)."""
    from contextlib import ExitStack

    import concourse.mybir as mybir
    import concourse.tile as tile
    from concourse import bacc

    f16 = mybir.dt.float16
    f32 = mybir.dt.float32
    f8 = mybir.dt.float8e4
    AF = mybir.ActivationFunctionType
    DR = mybir.MatmulPerfMode.DoubleRow

    T = H_ // 128       # hidden contraction tiles (fp16)
    T2 = H_ // 256      # hidden contraction tiles (fp8 DoubleRow)
    KT = S_ // 128      # key/seq tiles
    CC = S_ // 512      # query chunks
    HC = H_ // 512      # output hidden chunks
    qscale = (FP8_HSCALE * FP8_WSCALE) if use_fp8 else 1.0
    EXP_SCALE = 1.0 / (float(np.sqrt(HD)) * qscale * qscale)

    nc = bacc.Bacc("TRN2", target_bir_lowering=False, debug=False)

    hT_d = nc.dram_tensor("hT", [T, 128, S_], f16, kind="ExternalInput").ap()
    if use_fp8:
        h8_d = nc.dram_tensor("h8", [T2, 128, 2 * S_], f8, kind="ExternalInput").ap()
        wq_d = nc.dram_tensor("wq", [NHC_, 128, T2 * 2 * HD], f8, kind="ExternalInput").ap()
        wk_d = nc.dram_tensor("wk", [NHC_, 128, T2 * 2 * HD], f8, kind="ExternalInput").ap()
    else:
        wq_d = nc.dram_tensor("wq", [NHC_, 128, T * HD], f16, kind="ExternalInput").ap()
        wk_d = nc.dram_tensor("wk", [NHC_, 128, T * HD], f16, kind="ExternalInput").ap()
    # wvT/woT are pre-transposed host-side: contiguous [128, ...] DMAs
    wv_d = nc.dram_tensor("wvT", [128, T * NHC_ * HD], f16, kind="ExternalInput").ap()
    wo_d = nc.dram_tensor("woT", [128, NHC_ * H_], f16, kind="ExternalInput").ap()
    cos_d = nc.dram_tensor("cosT", [128, S_], f16, kind="ExternalInput").ap()
    ss_d = nc.dram_tensor("ssT", [128, S_], f16, kind="ExternalInput").ap()
    bq_d = nc.dram_tensor("bqT", [128, NHC_], f32, kind="ExternalInput").ap()
    bk_d = nc.dram_tensor("bkT", [128, NHC_], f32, kind="ExternalInput").ap()
    bv_d = nc.dram_tensor("bv4", [1, NHC_ * HD], f16, kind="ExternalInput").ap()
    # rotate-half permutation: swap[p, m] = 1 iff p == (m+64) % 128
    swap_d = nc.dram_tensor("swapT", [128, 128], f16, kind="ExternalInput").ap()
    # shared causal diagonal block: [128, i*512+q] for i = kt - 4c in [0,4)
    md_d = nc.dram_tensor("mdiag", [128, 4 * 512], f16, kind="ExternalInput").ap()
    o_d = nc.dram_tensor("o", [S_, H_], f16, kind="ExternalOutput").ap()

    with ExitStack() as ctx:
        tc = ctx.enter_context(tile.TileContext(nc))
        persist = ctx.enter_context(tc.tile_pool(name="persist", bufs=1))

        qr = [persist.tile([128, S_], f16, name=f"qr{j}") for j in range(NHC_)]
        kr = [persist.tile([128, S_], f16, name=f"kr{j}") for j in range(NHC_)]
        vs = persist.tile([128, KT * NHC_ * HD], f16, name="vs")
        wo_sb = persist.tile([128, NHC_ * H_], f16, name="wo_sb")
        ones_sb = persist.tile([128, 128], f16, name="ones_sb")
        ones1 = persist.tile([1, 128], f16, name="ones1")
        bv_sb = persist.tile([1, NHC_ * HD], f16, name="bv_sb")
        bq_sb = persist.tile([128, NHC_], f32, name="bq_sb")
        bk_sb = persist.tile([128, NHC_], f32, name="bk_sb")
        swap_sb = persist.tile([128, 128], f16, name="swap_sb")

        nc.vector.memset(ones_sb, 1.0)
        nc.vector.memset(ones1, 1.0)

        # ---------------- phase P: q/k/v projections + rope ----------------
        with tc.tile_pool(name="projp", bufs=1) as projp, \
             tc.tile_pool(name="projw", bufs=4) as projw, \
             tc.tile_pool(name="projtmp", bufs=2) as projtmp:

            # first two heads' weights lead the sync ring so PE starts asap;
            # the very first tile is split so its leading half lands sooner
            wq_sbs = {}
            for j0 in range(2):
                for w_d0, nm in ((wq_d, "q"), (wk_d, "k")):
                    wt = projw.tile(
                        [128, T2 * 2 * HD] if use_fp8 else [128, T * HD],
                        f8 if use_fp8 else f16, tag="w_sb", name=f"w{j0}{nm}")
                    if j0 == 0:
                        half = wt.shape[-1] // 2
                        nc.sync.dma_start(out=wt[:, :half], in_=w_d0[j0][:, :half])
                        nc.sync.dma_start(out=wt[:, half:], in_=w_d0[j0][:, half:])
                    else:
                        nc.sync.dma_start(out=wt, in_=w_d0[j0])
                    wq_sbs[(j0, nm)] = wt

            cos_sb = projp.tile([128, S_], f16, name="cos_sb")
            ss_sb = projp.tile([128, S_], f16, name="ss_sb")
            nc.sync.dma_start(out=swap_sb, in_=swap_d)
            nc.sync.dma_start(out=cos_sb, in_=cos_d)
            nc.sync.dma_start(out=ss_sb, in_=ss_d)
            nc.sync.dma_start(out=bv_sb, in_=bv_d)
            nc.sync.dma_start(out=bq_sb, in_=bq_d)
            nc.sync.dma_start(out=bk_sb, in_=bk_d)

            # bulk loads split across both DMA rings, halves per tile so the
            # leading query chunks unblock early; h8 is c-major [p,c,i,512]
            if use_fp8:
                h8_sb = [projp.tile([128, CC, 2, 512], f8, name=f"h8_{t}")
                         for t in range(T2)]
                for t in range(T2):
                    eng = nc.gpsimd if t % 2 == 0 else nc.sync
                    flat = h8_sb[t].rearrange("p c i s -> p (c i s)")
                    eng.dma_start(out=flat[:, : S_], in_=h8_d[t][:, : S_])
                    eng.dma_start(out=flat[:, S_:], in_=h8_d[t][:, S_:])
            wv_sb = projp.tile([128, T * NHC_ * HD], f16, name="wv_sb")
            nc.sync.dma_start(out=wv_sb, in_=wv_d)
            hT_sb = [projp.tile([128, S_], f16, name=f"hT{t}") for t in range(T)]
            for t in range(T):
                nc.gpsimd.dma_start(out=hT_sb[t], in_=hT_d[t])

            # q^T / k^T per head: [d=128, s] = sum_t Wx[t]^T @ hT[t]
            with tc.tile_pool(name="pps", bufs=1, space="PSUM") as pps:
                for j in range(NHC_):
                    for (w_d, nm, b_sb, dest) in ((wq_d, "q", bq_sb, qr[j]),
                                                  (wk_d, "k", bk_sb, kr[j])):
                        if j < 2:
                            w_sb = wq_sbs[(j, nm)]
                        else:
                            w_sb = projw.tile(
                                [128, T2 * 2 * HD] if use_fp8 else [128, T * HD],
                                f8 if use_fp8 else f16, tag="w_sb", name="w_sb")
                            nc.sync.dma_start(out=w_sb, in_=w_d[j])
                        qs = projtmp.tile([128, S_], f16, tag="qs", name="qs")
                        # t-outer with 4 live PSUM groups: matmuls consume
                        # the hidden tiles in DMA-arrival order
                        pp = [pps.tile([128, 512], f32, tag="pp", bufs=6,
                                       name="pp") for _ in range(CC)]
                        if use_fp8:
                            w3 = w_sb.rearrange("p (t i m) -> p t i m",
                                                t=T2, i=2)
                            for t in range(T2):
                                for c in range(CC):
                                    nc.tensor.matmul(
                                        pp[c],
                                        lhsT=w3[:, t],
                                        rhs=h8_sb[t][:, c],
                                        start=(t == 0), stop=(t == T2 - 1),
                                        perf_mode=DR)
                        else:
                            for t in range(T):
                                for c in range(CC):
                                    nc.tensor.matmul(
                                        pp[c],
                                        lhsT=w_sb[:, t * HD:(t + 1) * HD],
                                        rhs=hT_sb[t][:, c * 512: c * 512 + 512],
                                        start=(t == 0), stop=(t == T - 1))
                        for c in range(CC):
                            # evacuate + bias (per-partition, pre-scaled) on
                            # the (projection-idle) scalar engine
                            nc.scalar.activation(
                                qs[:, c * 512:(c + 1) * 512], pp[c],
                                AF.Identity, bias=b_sb[:, j:j + 1])
                        # rope: dest = qs*cos + shift(qs)*ss. The rotate-half
                        # partition swap runs on the PE as a permutation
                        # matmul; vector reads the swapped copy from PSUM.
                        for c in range(CC):
                            sl = slice(c * 512, (c + 1) * 512)
                            rot = pps.tile([128, 512], f32, tag="rot", bufs=2,
                                           name="rot")
                            nc.tensor.matmul(rot, lhsT=swap_sb,
                                             rhs=qs[:, sl],
                                             start=True, stop=True)
                            qshc = projtmp.tile([128, 512], f16, tag="qshc",
                                                name="qshc")
                            nc.vector.tensor_mul(dest[:, sl], qs[:, sl],
                                                 cos_sb[:, sl])
                            nc.vector.tensor_mul(qshc, rot, ss_sb[:, sl])
                            nc.vector.tensor_add(dest[:, sl], dest[:, sl],
                                                 qshc)

            # v: [s, (j d)] = sum_t hT[t, s-tile]^T @ Wv[t]; t-outer over two
            # 8-tile PSUM passes so matmuls chase the hT DMAs
            with tc.tile_pool(name="vps", bufs=1, space="PSUM") as vps:
                for half in range(2):
                    vp = [vps.tile([128, NHC_ * HD], f32, tag=f"vp{st}",
                                   name=f"vp{st}") for st in range(8)]
                    for t in range(T):
                        for sti in range(8):
                            nc.tensor.matmul(
                                vp[sti],
                                lhsT=hT_sb[t][:, (half * 8 + sti) * 128:
                                              (half * 8 + sti) * 128 + 128],
                                rhs=wv_sb[:, t * NHC_ * HD:(t + 1) * NHC_ * HD],
                                start=(t == 0), stop=False)
                    for sti in range(8):
                        st = half * 8 + sti
                        nc.tensor.matmul(vp[sti], lhsT=ones1, rhs=bv_sb,
                                         start=False, stop=True)
                        nc.vector.tensor_copy(
                            vs[:, st * NHC_ * HD:(st + 1) * NHC_ * HD], vp[sti])

        # ---------------- phase A: attention + output projection ----------------
        with tc.tile_pool(name="attnp", bufs=2) as attnp, \
             tc.tile_pool(name="osbp", bufs=4) as osbp, \
             tc.tile_pool(name="aps", bufs=1, space="PSUM") as aps:

            nc.gpsimd.dma_start(out=wo_sb, in_=wo_d)
            md_sb = attnp.tile([128, 4 * 512], f16, tag="md", bufs=1, name="md_sb")
            nc.sync.dma_start(out=md_sb, in_=md_d)

            out_dma = 0
            for c in range(CC):
                inc = 4 * c + 4          # causal: kt in [0, 4c+4)
                P = inc // 2             # score pairs
                # per-head attnT tiles: out-proj reads of head j wait only on
                # head j's normalize, so heads 0-2 stream while 3 finishes
                attnTs = [attnp.tile([128, 512], f16, tag=f"attnT{j}",
                                     name=f"attnT{j}") for j in range(NHC_)]
                pending_ones = []

                def issue_ones(ent):
                    jj, dd, aa = ent
                    # partition-reduce + broadcast both den2 halves in one
                    # PSUM accumulation (no separate fold op)
                    lb = aps.tile([128, 512], f32, tag="lb", bufs=1, name="lb")
                    nc.tensor.matmul(lb, lhsT=ones_sb, rhs=dd[:, 0:512],
                                     start=True, stop=False)
                    nc.tensor.matmul(lb, lhsT=ones_sb, rhs=dd[:, 512:1024],
                                     start=False, stop=True)
                    rl = attnp.tile([128, 512], f32, tag="rl", name="rl")
                    nc.vector.reciprocal_approx_fast(rl, lb)
                    nc.vector.tensor_mul(attnTs[jj], aa, rl)

                # diagonal (masked) pairs first so the chunk tail ends on a
                # clean pair: the denominator chain after the last exp is
                # just one den2 add
                order = list(range(2 * c, P)) + list(range(2 * c))

                for j in range(NHC_):
                    expT = attnp.tile([128, KT * 512], f16, tag="expT",
                                      bufs=2, name="expT")
                    expT3 = expT.rearrange("p (t x) -> p t x", x=512)
                    den2 = attnp.tile([128, 1024], f16, tag="den2", bufs=2,
                                      name="den2")

                    # software pipeline: scores pair p, then AV of pair p-2;
                    # exp fuses the softmax scale; diagonal pairs multiply
                    # the shared causal mask block; den2 pair-sums feed the
                    # denominator
                    def scores(pi):
                        p = order[pi]
                        scp = aps.tile([128, 1024], f32, tag="scp", bufs=3,
                                       name="scp")
                        for kk in range(2):
                            kt = 2 * p + kk
                            i0 = max(kt - 4 * c, 0) if 2 * p >= 4 * c else 0
                            nc.tensor.matmul(
                                scp[:, kk * 512 + i0 * 128:(kk + 1) * 512],
                                lhsT=kr[j][:, kt * 128:(kt + 1) * 128],
                                rhs=qr[j][:, c * 512 + i0 * 128:(c + 1) * 512],
                                start=True, stop=True)
                        nc.scalar.activation(
                            expT[:, 2 * p * 512:(2 * p + 2) * 512],
                            scp, AF.Exp, scale=EXP_SCALE)
                        if 2 * p >= 4 * c:  # diagonal pair: mask multiply
                            i0 = 2 * p - 4 * c
                            sl = slice(2 * p * 512, (2 * p + 2) * 512)
                            nc.vector.tensor_mul(expT[:, sl], expT[:, sl],
                                                 md_sb[:, i0 * 512:(i0 + 2) * 512])
                        pair = expT[:, 2 * p * 512:(2 * p + 2) * 512]
                        if pi == 0:
                            nc.vector.tensor_copy(den2, pair)
                        else:
                            nc.vector.tensor_add(den2, den2, pair)

                    def av(pi, jau):
                        p = order[pi]
                        for kk in range(2):
                            kt = 2 * p + kk
                            i0 = max(kt - 4 * c, 0) if 2 * p >= 4 * c else 0
                            base = (kt * NHC_ + j) * HD
                            nc.tensor.matmul(
                                jau[:, i0 * 128:512],
                                lhsT=vs[:, base: base + HD],
                                rhs=expT3[:, kt][:, i0 * 128:512],
                                start=(pi == 0 and kk == 0),
                                stop=(pi == P - 1 and kk == 1))

                    au = aps.tile([128, 512], f32, tag="au", bufs=1, name="au")
                    for pi in range(min(3, P)):
                        scores(pi)
                        if pi == 1 and pending_ones:
                            issue_ones(pending_ones.pop())
                    for pi in range(3, P):
                        av(pi - 3, au)
                        scores(pi)
                    for pi in range(max(P - 3, 0), P):
                        av(pi, au)
                    pending_ones.append((j, den2, au))
                    if j == NHC_ - 1:
                        issue_ones(pending_ones.pop())

                # out-projection: accumulate the 4 local heads; [128,1024]
                # PSUM groups share the scp ring, two N=512 halves each
                for st in range(4):
                    for hcp in range(HC // 2):
                        op = aps.tile([128, 1024], f32, tag="scp", bufs=3,
                                      name="op")
                        for half in range(2):
                            for j in range(NHC_):
                                col = j * H_ + hcp * 1024 + half * 512
                                nc.tensor.matmul(
                                    op[:, half * 512:(half + 1) * 512],
                                    lhsT=attnTs[j][:, st * 128: st * 128 + 128],
                                    rhs=wo_sb[:, col: col + 512],
                                    start=(j == 0), stop=(j == NHC_ - 1))
                        osb = osbp.tile([128, 1024], f16, tag="osb", name="osb")
                        if hcp % 2:
                            nc.scalar.copy(osb, op)
                        else:
                            nc.vector.tensor_copy(osb, op)
                        row = c * 512 + st * 128
                        eng = nc.sync if out_dma % 2 else nc.gpsimd
                        out_dma += 1
                        eng.dma_start(
                            out=o_d[row: row + 128, hcp * 1024:(hcp + 1) * 1024],
                            in_=osb)

    nc.compile()
    return nc


def prep_core_inputs(hidden_b, mask_b, Wq, bq, Wk, bk, Wv, bv, Wo, n0, S_, H_, NHC_,
                     cosT, ssT, use_fp8):
    """Host-side prep of one core's input map. hidden_b [S,H] f32, mask_b [S,S]."""
    import ml_dtypes

    T = H_ // 128
    T2 = H_ // 256
    f16 = np.float16
    f8 = ml_dtypes.float8_e4m3

    hT = np.ascontiguousarray(hidden_b.T).reshape(T, 128, S_).astype(f16)

    inp = {"hT": hT, "cosT": cosT, "ssT": ssT}

    if use_fp8:
        # h8: c-major [T2, 128, CC, 2, 512] with h = t2*256 + p*2 + i
        CCl = S_ // 512
        h8 = np.clip(hidden_b.T * FP8_HSCALE, -240, 240).astype(f8)
        h8 = h8.reshape(T2, 128, 2, CCl, 512).transpose(0, 1, 3, 2, 4)
        inp["h8"] = np.ascontiguousarray(h8).reshape(T2, 128, 2 * S_)

        def w_slices8(W):
            out = np.empty((NHC_, 128, T2 * 2 * HD), f8)
            for j in range(NHC_):
                w = np.clip(W[:, n0 + j, :] * FP8_WSCALE, -240, 240).astype(f8)
                w = w.reshape(T2, 128, 2, HD)          # [t2, p, i, d]
                out[j] = w.transpose(1, 0, 2, 3).reshape(128, T2 * 2 * HD)
            return out

        inp["wq"] = w_slices8(Wq)
        inp["wk"] = w_slices8(Wk)
        bscale = FP8_HSCALE * FP8_WSCALE
    else:
        def w_slices(W):
            out = np.empty((NHC_, 128, T * HD), f16)
            for j in range(NHC_):
                w = W[:, n0 + j, :].reshape(T, 128, HD)     # [t, p, d]
                out[j] = w.transpose(1, 0, 2).reshape(128, T * HD)
            return out

        inp["wq"] = w_slices(Wq)
        inp["wk"] = w_slices(Wk)
        bscale = 1.0

    # [t, p, x] -> [p, (t x)] so the device DMA is contiguous
    wvt = np.ascontiguousarray(
        Wv[:, n0:n0 + NHC_, :]).reshape(T, 128, NHC_ * HD).astype(f16)
    inp["wvT"] = np.ascontiguousarray(wvt.transpose(1, 0, 2)).reshape(
        128, T * NHC_ * HD)
    # [j, p, h] -> [p, (j h)]
    wot = np.ascontiguousarray(Wo[n0:n0 + NHC_]).astype(f16)
    inp["woT"] = np.ascontiguousarray(wot.transpose(1, 0, 2)).reshape(
        128, NHC_ * H_)

    inp["bqT"] = np.ascontiguousarray(bq[n0:n0 + NHC_].T * bscale).astype(np.float32)
    inp["bkT"] = np.ascontiguousarray(bk[n0:n0 + NHC_].T * bscale).astype(np.float32)
    inp["bv4"] = bv[n0:n0 + NHC_].reshape(1, NHC_ * HD).astype(f16)

    swap = np.zeros((128, 128), f16)
    m_idx = np.arange(128)
    swap[(m_idx + 64) % 128, m_idx] = 1.0
    inp["swapT"] = swap

    # causal mask checks + shared diagonal block [128, i*512 + q], i = kt-4c
    m01 = (mask_b <= 0.5).astype(np.float32).T      # [k, q] keep-mask
    KT, CC = S_ // 128, S_ // 512
    m4 = m01.reshape(KT, 128, CC, 512)              # [kt, p, c, q]
    mdiag = None
    for c in range(CC):
        blk = m4[4 * c:4 * c + 4, :, c, :]          # [4, 128, 512]
        if mdiag is None:
            mdiag = blk
        else:
            assert np.array_equal(blk, mdiag), "mask diagonal blocks differ"
        assert m4[: 4 * c, :, c, :].all(), "mask below diagonal not all-keep"
        assert not m4[4 * c + 4:, :, c, :].any(), "mask above diagonal not all-drop"
    inp["mdiag"] = np.ascontiguousarray(
        mdiag.transpose(1, 0, 2)).reshape(128, 4 * 512).astype(f16)
    return inp


def kernel(hidden_states, mask, Wq, bq, Wk, bk, Wv, bv, Wo, bo):
    global LAST_RESULTS
    from concourse.bass_utils import run_bass_kernel_spmd

    hidden_states = np.asarray(hidden_states, dtype=np.float32)
    mask = np.asarray(mask, dtype=np.float32)
    Wq, bq = np.asarray(Wq, np.float32), np.asarray(bq, np.float32)
    Wk, bk = np.asarray(Wk, np.float32), np.asarray(bk, np.float32)
    Wv, bv = np.asarray(Wv, np.float32), np.asarray(bv, np.float32)
    Wo, bo = np.asarray(Wo, np.float32), np.asarray(bo, np.float32)

    cosT, ssT = _rope_tables(S)
    in_maps = []
    for core in range(N_CORES):
        b = core // HGRID
        n0 = (core % HGRID) * NHC
        in_maps.append(prep_core_inputs(
            hidden_states[b], mask[b, 0], Wq, bq, Wk, bk, Wv, bv, Wo,
            n0, S, H, NHC, cosT, ssT, USE_FP8))

    key = (S, H, NHC, USE_FP8)
    if key not in _CACHE:
        _CACHE[key] = build_program(S, H, NHC, USE_FP8)
    nc = _CACHE[key]

    res = run_bass_kernel_spmd(nc, in_maps, core_ids=list(range(N_CORES)))
    LAST_RESULTS = res

    out = np.zeros((B, S, H), np.float32)
    for core in range(N_CORES):
        out[core // HGRID] += res.results[core]["o"].astype(np.float32)
    out += bo[None, None, :]
    return out


# revision 23
# speedup vs baseline: 1.0667x; 1.0667x over previous
"""Trainium2 Bass kernel for multi-head self-attention with RoPE.

Sharding: 8 cores = 2 (batch) x 4 (head groups of 4 heads).
Each core computes its batch's attention for its 4 heads plus the
(row-sharded) output projection partial sum; the host adds the 4 head-group
partials per batch and the output bias.

Structure (v3):
- Q/K projections run in fp8(e4m3) DoubleRow mode (2x PE throughput); the
  4096x operand scaling is undone inside the exp() activation scale.
- V projection is t-outer over two 8-column-tile PSUM passes so it consumes
  the fp16 hidden tiles as their DMAs land.
- Attention per (chunk, head): score pairs -> one [128,1024] exp activation
  (scale fused) -> AV matmuls interleaved two pairs behind; the softmax
  denominator is pair-accumulated on vector, partition-reduced+broadcast by
  a single ones-matmul, inverted with reciprocal_approx_fast.
- Output projection: per-[128,512] PSUM groups, evacuation alternating
  scalar/vector, fp16 DMA alternating sync/gpsimd queues.
- Causal mask: one shared diagonal 512x512 block, multiplied into the two
  diagonal exp pairs; off-diagonal tiles are skipped (upper) or clean (lower).
"""

import sys
import types

import numpy as np

sys.path.insert(0, "/opt/trn_rl_repo")

# The axon boot registers its NTFF-profiling hook via antenv.axon_hooks; some
# images lack that module, which silently disables tracing. Provide it.
if "antenv.axon_hooks" not in sys.modules:
    try:
        import antenv.axon_hooks  # noqa: F401
    except ImportError:
        try:
            import antenv

            _m = types.ModuleType("antenv.axon_hooks")
            _m._hook = None
            _m.set_axon_ntff_profile_hook = lambda h: setattr(_m, "_hook", h)
            _m.get_axon_ntff_profile_hook = lambda: _m._hook
            sys.modules["antenv.axon_hooks"] = _m
            antenv.axon_hooks = _m
        except ImportError:
            pass

B, S, H, NH, HD = 2, 2048, 2048, 16, 128
ROPE_THETA = 10000.0
N_CORES = 8
HGRID = 4            # head-group shards
NHC = NH // HGRID    # heads per core

USE_FP8 = True       # fp8 e4m3 DoubleRow for the Q/K projections
FP8_HSCALE = 16.0    # hidden scale into fp8 (power of 2)
FP8_WSCALE = 256.0   # weight scale into fp8 (power of 2)

LAST_RESULTS = None  # test harness introspection
_CACHE = {}


def _rope_tables(S_, dtype=np.float16):
    # transposed rope tables [HD, S]; ss has rotate-half sign folded in:
    # rope(x)[d, s] = x[d, s]*cosT[d, s] + x[(d+64)%128, s]*ss[d, s]
    inv = 1.0 / (ROPE_THETA ** (np.arange(0, HD, 2, dtype=np.float64) / HD))
    t = np.arange(S_, dtype=np.float64)
    fr = np.outer(t, inv)                          # [S, HD/2]
    emb = np.concatenate([fr, fr], axis=1)         # [S, HD]
    cosT = np.cos(emb).T.astype(np.float32)        # [HD, S]
    ss = np.sin(emb).T.astype(np.float32)
    ss[: HD // 2] *= -1.0
    return cosT.astype(dtype), ss.astype(dtype)


def build_program(S_, H_, NHC_, use_fp8):
    """Build + compile the per-core SPMD bass program (causal mask layout)."""
    from contextlib import ExitStack

    import concourse.mybir as mybir
    import concourse.tile as tile
    from concourse import bacc

    f16 = mybir.dt.float16
    f32 = mybir.dt.float32
    f8 = mybir.dt.float8e4
    AF = mybir.ActivationFunctionType
    DR = mybir.MatmulPerfMode.DoubleRow

    T = H_ // 128       # hidden contraction tiles (fp16)
    T2 = H_ // 256      # hidden contraction tiles (fp8 DoubleRow)
    KT = S_ // 128      # key/seq tiles
    CC = S_ // 512      # query chunks
    HC = H_ // 512      # output hidden chunks
    qscale = (FP8_HSCALE * FP8_WSCALE) if use_fp8 else 1.0
    EXP_SCALE = 1.0 / (float(np.sqrt(HD)) * qscale * qscale)

    nc = bacc.Bacc("TRN2", target_bir_lowering=False, debug=False)

    hT_d = nc.dram_tensor("hT", [T, 128, S_], f16, kind="ExternalInput").ap()
    if use_fp8:
        h8_d = nc.dram_tensor("h8", [T2, 128, 2 * S_], f8, kind="ExternalInput").ap()
        wq_d = nc.dram_tensor("wq", [NHC_, 128, T2 * 2 * HD], f8, kind="ExternalInput").ap()
        wk_d = nc.dram_tensor("wk", [NHC_, 128, T2 * 2 * HD], f8, kind="ExternalInput").ap()
    else:
        wq_d = nc.dram_tensor("wq", [NHC_, 128, T * HD], f16, kind="ExternalInput").ap()
        wk_d = nc.dram_tensor("wk", [NHC_, 128, T * HD], f16, kind="ExternalInput").ap()
    # wvT/woT are pre-transposed host-side: contiguous [128, ...] DMAs
    wv_d = nc.dram_tensor("wvT", [128, T * NHC_ * HD], f16, kind="ExternalInput").ap()
    wo_d = nc.dram_tensor("woT", [128, NHC_ * H_], f16, kind="ExternalInput").ap()
    cos_d = nc.dram_tensor("cosT", [128, S_], f16, kind="ExternalInput").ap()
    ss_d = nc.dram_tensor("ssT", [128, S_], f16, kind="ExternalInput").ap()
    bq_d = nc.dram_tensor("bqT", [128, NHC_], f32, kind="ExternalInput").ap()
    bk_d = nc.dram_tensor("bkT", [128, NHC_], f32, kind="ExternalInput").ap()
    bv_d = nc.dram_tensor("bv4", [1, NHC_ * HD], f16, kind="ExternalInput").ap()
    # rotate-half permutation: swap[p, m] = 1 iff p == (m+64) % 128
    swap_d = nc.dram_tensor("swapT", [128, 128], f16, kind="ExternalInput").ap()
    # shared causal diagonal block: [128, i*512+q] for i = kt - 4c in [0,4)
    md_d = nc.dram_tensor("mdiag", [128, 4 * 512], f16, kind="ExternalInput").ap()
    o_d = nc.dram_tensor("o", [S_, H_], f16, kind="ExternalOutput").ap()

    with ExitStack() as ctx:
        tc = ctx.enter_context(tile.TileContext(nc))
        persist = ctx.enter_context(tc.tile_pool(name="persist", bufs=1))

        qr = [persist.tile([128, S_], f16, name=f"qr{j}") for j in range(NHC_)]
        kr = [persist.tile([128, S_], f16, name=f"kr{j}") for j in range(NHC_)]
        vs = persist.tile([128, KT * NHC_ * HD], f16, name="vs")
        wo_sb = persist.tile([128, NHC_ * H_], f16, name="wo_sb")
        ones_sb = persist.tile([128, 128], f16, name="ones_sb")
        ones1 = persist.tile([1, 128], f16, name="ones1")
        bv_sb = persist.tile([1, NHC_ * HD], f16, name="bv_sb")
        bq_sb = persist.tile([128, NHC_], f32, name="bq_sb")
        bk_sb = persist.tile([128, NHC_], f32, name="bk_sb")
        swap_sb = persist.tile([128, 128], f16, name="swap_sb")

        nc.vector.memset(ones_sb, 1.0)
        nc.vector.memset(ones1, 1.0)

        # ---------------- phase P: q/k/v projections + rope ----------------
        with tc.tile_pool(name="projp", bufs=1) as projp, \
             tc.tile_pool(name="projw", bufs=4) as projw, \
             tc.tile_pool(name="projtmp", bufs=2) as projtmp:

            # first two heads' weights lead the sync ring so PE starts asap;
            # the very first tile is split so its leading half lands sooner
            wq_sbs = {}
            for j0 in range(2):
                for w_d0, nm in ((wq_d, "q"), (wk_d, "k")):
                    wt = projw.tile(
                        [128, T2 * 2 * HD] if use_fp8 else [128, T * HD],
                        f8 if use_fp8 else f16, tag="w_sb", name=f"w{j0}{nm}")
                    if j0 == 0:
                        half = wt.shape[-1] // 2
                        nc.sync.dma_start(out=wt[:, :half], in_=w_d0[j0][:, :half])
                        nc.sync.dma_start(out=wt[:, half:], in_=w_d0[j0][:, half:])
                    else:
                        nc.sync.dma_start(out=wt, in_=w_d0[j0])
                    wq_sbs[(j0, nm)] = wt

            cos_sb = projp.tile([128, S_], f16, name="cos_sb")
            ss_sb = projp.tile([128, S_], f16, name="ss_sb")
            nc.sync.dma_start(out=swap_sb, in_=swap_d)
            nc.sync.dma_start(out=cos_sb, in_=cos_d)
            nc.sync.dma_start(out=ss_sb, in_=ss_d)
            nc.sync.dma_start(out=bv_sb, in_=bv_d)
            nc.sync.dma_start(out=bq_sb, in_=bq_d)
            nc.sync.dma_start(out=bk_sb, in_=bk_d)

            # bulk loads split across both DMA rings, halves per tile so the
            # leading query chunks unblock early; h8 is c-major [p,c,i,512]
            if use_fp8:
                h8_sb = [projp.tile([128, CC, 2, 512], f8, name=f"h8_{t}")
                         for t in range(T2)]
                for t in range(T2):
                    flat = h8_sb[t].rearrange("p c i s -> p (c i s)")
                    nc.gpsimd.dma_start(out=flat[:, : S_], in_=h8_d[t][:, : S_])
                    nc.gpsimd.dma_start(out=flat[:, S_:], in_=h8_d[t][:, S_:])
            wv_sb = projp.tile([128, T * NHC_ * HD], f16, name="wv_sb")
            nc.gpsimd.dma_start(out=wv_sb, in_=wv_d)
            hT_sb = [projp.tile([128, S_], f16, name=f"hT{t}") for t in range(T)]
            for t in range(T):
                nc.gpsimd.dma_start(out=hT_sb[t], in_=hT_d[t])

            # q^T / k^T per head: [d=128, s] = sum_t Wx[t]^T @ hT[t]
            with tc.tile_pool(name="pps", bufs=1, space="PSUM") as pps:
                for j in range(NHC_):
                    for (w_d, nm, b_sb, dest) in ((wq_d, "q", bq_sb, qr[j]),
                                                  (wk_d, "k", bk_sb, kr[j])):
                        if j < 2:
                            w_sb = wq_sbs[(j, nm)]
                        else:
                            w_sb = projw.tile(
                                [128, T2 * 2 * HD] if use_fp8 else [128, T * HD],
                                f8 if use_fp8 else f16, tag="w_sb", name="w_sb")
                            nc.sync.dma_start(out=w_sb, in_=w_d[j])
                        qs = projtmp.tile([128, S_], f16, tag="qs", name="qs")
                        # t-outer with 4 live PSUM groups: matmuls consume
                        # the hidden tiles in DMA-arrival order
                        pp = [pps.tile([128, 512], f32, tag="pp", bufs=6,
                                       name="pp") for _ in range(CC)]
                        if use_fp8:
                            w3 = w_sb.rearrange("p (t i m) -> p t i m",
                                                t=T2, i=2)
                            for t in range(T2):
                                for c in range(CC):
                                    nc.tensor.matmul(
                                        pp[c],
                                        lhsT=w3[:, t],
                                        rhs=h8_sb[t][:, c],
                                        start=(t == 0), stop=(t == T2 - 1),
                                        perf_mode=DR)
                        else:
                            for t in range(T):
                                for c in range(CC):
                                    nc.tensor.matmul(
                                        pp[c],
                                        lhsT=w_sb[:, t * HD:(t + 1) * HD],
                                        rhs=hT_sb[t][:, c * 512: c * 512 + 512],
                                        start=(t == 0), stop=(t == T - 1))
                        for c in range(CC):
                            # evacuate + bias (per-partition, pre-scaled) on
                            # the (projection-idle) scalar engine
                            nc.scalar.activation(
                                qs[:, c * 512:(c + 1) * 512], pp[c],
                                AF.Identity, bias=b_sb[:, j:j + 1])
                        # rope: dest = qs*cos + shift(qs)*ss. The rotate-half
                        # partition swap runs on the PE as a permutation
                        # matmul; vector reads the swapped copy from PSUM.
                        for c in range(CC):
                            sl = slice(c * 512, (c + 1) * 512)
                            rot = pps.tile([128, 512], f32, tag="rot", bufs=2,
                                           name="rot")
                            nc.tensor.matmul(rot, lhsT=swap_sb,
                                             rhs=qs[:, sl],
                                             start=True, stop=True)
                            qshc = projtmp.tile([128, 512], f16, tag="qshc",
                                                name="qshc")
                            nc.vector.tensor_mul(dest[:, sl], qs[:, sl],
                                                 cos_sb[:, sl])
                            nc.vector.tensor_mul(qshc, rot, ss_sb[:, sl])
                            nc.vector.tensor_add(dest[:, sl], dest[:, sl],
                                                 qshc)

            # v: [s, (j d)] = sum_t hT[t, s-tile]^T @ Wv[t]; t-outer over two
            # 8-tile PSUM passes so matmuls chase the hT DMAs
            with tc.tile_pool(name="vps", bufs=1, space="PSUM") as vps:
                for half in range(2):
                    vp = [vps.tile([128, NHC_ * HD], f32, tag=f"vp{st}",
                                   name=f"vp{st}") for st in range(8)]
                    for t in range(T):
                        for sti in range(8):
                            nc.tensor.matmul(
                                vp[sti],
                                lhsT=hT_sb[t][:, (half * 8 + sti) * 128:
                                              (half * 8 + sti) * 128 + 128],
                                rhs=wv_sb[:, t * NHC_ * HD:(t + 1) * NHC_ * HD],
                                start=(t == 0), stop=False)
                    for sti in range(8):
                        st = half * 8 + sti
                        nc.tensor.matmul(vp[sti], lhsT=ones1, rhs=bv_sb,
                                         start=False, stop=True)
                        nc.vector.tensor_copy(
                            vs[:, st * NHC_ * HD:(st + 1) * NHC_ * HD], vp[sti])

        # ---------------- phase A: attention + output projection ----------------
        with tc.tile_pool(name="attnp", bufs=2) as attnp, \
             tc.tile_pool(name="osbp", bufs=4) as osbp, \
             tc.tile_pool(name="aps", bufs=1, space="PSUM") as aps:

            nc.gpsimd.dma_start(out=wo_sb, in_=wo_d)
            md_sb = attnp.tile([128, 4 * 512], f16, tag="md", bufs=1, name="md_sb")
            nc.sync.dma_start(out=md_sb, in_=md_d)

            out_dma = 0
            for c in range(CC):
                inc = 4 * c + 4          # causal: kt in [0, 4c+4)
                P = inc // 2             # score pairs
                # per-head attnT tiles: out-proj reads of head j wait only on
                # head j's normalize, so heads 0-2 stream while 3 finishes
                attnTs = [attnp.tile([128, 512], f16, tag=f"attnT{j}",
                                     name=f"attnT{j}") for j in range(NHC_)]
                pending_ones = []

                def issue_ones(ent):
                    jj, dd, aa = ent
                    # partition-reduce + broadcast both den2 halves in one
                    # PSUM accumulation (no separate fold op)
                    lb = aps.tile([128, 512], f32, tag="lb", bufs=1, name="lb")
                    nc.tensor.matmul(lb, lhsT=ones_sb, rhs=dd[:, 0:512],
                                     start=True, stop=False)
                    nc.tensor.matmul(lb, lhsT=ones_sb, rhs=dd[:, 512:1024],
                                     start=False, stop=True)
                    rl = attnp.tile([128, 512], f32, tag="rl", name="rl")
                    nc.vector.reciprocal_approx_fast(rl, lb)
                    nc.vector.tensor_mul(attnTs[jj], aa, rl)

                # diagonal (masked) pairs first so the chunk tail ends on a
                # clean pair: the denominator chain after the last exp is
                # just one den2 add
                order = list(range(2 * c, P)) + list(range(2 * c))

                for j in range(NHC_):
                    expT = attnp.tile([128, KT * 512], f16, tag="expT",
                                      bufs=2, name="expT")
                    expT3 = expT.rearrange("p (t x) -> p t x", x=512)
                    den2 = attnp.tile([128, 1024], f16, tag="den2", bufs=2,
                                      name="den2")

                    # software pipeline: scores pair p, then AV of pair p-2;
                    # exp fuses the softmax scale; diagonal pairs multiply
                    # the shared causal mask block; den2 pair-sums feed the
                    # denominator
                    def scores(pi):
                        p = order[pi]
                        scp = aps.tile([128, 1024], f32, tag="scp", bufs=3,
                                       name="scp")
                        for kk in range(2):
                            kt = 2 * p + kk
                            i0 = max(kt - 4 * c, 0) if 2 * p >= 4 * c else 0
                            nc.tensor.matmul(
                                scp[:, kk * 512 + i0 * 128:(kk + 1) * 512],
                                lhsT=kr[j][:, kt * 128:(kt + 1) * 128],
                                rhs=qr[j][:, c * 512 + i0 * 128:(c + 1) * 512],
                                start=True, stop=True)
                        nc.scalar.activation(
                            expT[:, 2 * p * 512:(2 * p + 2) * 512],
                            scp, AF.Exp, scale=EXP_SCALE)
                        if 2 * p >= 4 * c:  # diagonal pair: mask multiply
                            i0 = 2 * p - 4 * c
                            sl = slice(2 * p * 512, (2 * p + 2) * 512)
                            nc.vector.tensor_mul(expT[:, sl], expT[:, sl],
                                                 md_sb[:, i0 * 512:(i0 + 2) * 512])
                        pair = expT[:, 2 * p * 512:(2 * p + 2) * 512]
                        if pi == 0:
                            nc.vector.tensor_copy(den2, pair)
                        else:
                            nc.vector.tensor_add(den2, den2, pair)

                    def av(pi, jau):
                        p = order[pi]
                        for kk in range(2):
                            kt = 2 * p + kk
                            i0 = max(kt - 4 * c, 0) if 2 * p >= 4 * c else 0
                            base = (kt * NHC_ + j) * HD
                            nc.tensor.matmul(
                                jau[:, i0 * 128:512],
                                lhsT=vs[:, base: base + HD],
                                rhs=expT3[:, kt][:, i0 * 128:512],
                                start=(pi == 0 and kk == 0),
                                stop=(pi == P - 1 and kk == 1))

                    au = aps.tile([128, 512], f32, tag="au", bufs=1, name="au")
                    for pi in range(min(3, P)):
                        scores(pi)
                        if pi == 1 and pending_ones:
                            issue_ones(pending_ones.pop())
                    for pi in range(3, P):
                        av(pi - 3, au)
                        scores(pi)
                    for pi in range(max(P - 3, 0), P):
                        av(pi, au)
                    pending_ones.append((j, den2, au))
                    if j == NHC_ - 1:
                        issue_ones(pending_ones.pop())

                # out-projection: accumulate the 4 local heads; [128,1024]
                # PSUM groups share the scp ring, two N=512 halves each
                for st in range(4):
                    for hcp in range(HC // 2):
                        op = aps.tile([128, 1024], f32, tag="scp", bufs=3,
                                      name="op")
                        for half in range(2):
                            for j in range(NHC_):
                                col = j * H_ + hcp * 1024 + half * 512
                                nc.tensor.matmul(
                                    op[:, half * 512:(half + 1) * 512],
                                    lhsT=attnTs[j][:, st * 128: st * 128 + 128],
                                    rhs=wo_sb[:, col: col + 512],
                                    start=(j == 0), stop=(j == NHC_ - 1))
                        osb = osbp.tile([128, 1024], f16, tag="osb", name="osb")
                        if hcp % 2:
                            nc.scalar.copy(osb, op)
                        else:
                            nc.vector.tensor_copy(osb, op)
                        row = c * 512 + st * 128
                        eng = nc.sync if out_dma % 2 else nc.gpsimd
                        out_dma += 1
                        eng.dma_start(
                            out=o_d[row: row + 128, hcp * 1024:(hcp + 1) * 1024],
                            in_=osb)

    nc.compile()
    return nc


def prep_core_inputs(hidden_b, mask_b, Wq, bq, Wk, bk, Wv, bv, Wo, n0, S_, H_, NHC_,
                     cosT, ssT, use_fp8):
    """Host-side prep of one core's input map. hidden_b [S,H] f32, mask_b [S,S]."""
    import ml_dtypes

    T = H_ // 128
    T2 = H_ // 256
    f16 = np.float16
    f8 = ml_dtypes.float8_e4m3

    hT = np.ascontiguousarray(hidden_b.T).reshape(T, 128, S_).astype(f16)

    inp = {"hT": hT, "cosT": cosT, "ssT": ssT}

    if use_fp8:
        # h8: c-major [T2, 128, CC, 2, 512] with h = t2*256 + p*2 + i
        CCl = S_ // 512
        h8 = np.clip(hidden_b.T * FP8_HSCALE, -240, 240).astype(f8)
        h8 = h8.reshape(T2, 128, 2, CCl, 512).transpose(0, 1, 3, 2, 4)
        inp["h8"] = np.ascontiguousarray(h8).reshape(T2, 128, 2 * S_)

        def w_slices8(W):
            out = np.empty((NHC_, 128, T2 * 2 * HD), f8)
            for j in range(NHC_):
                w = np.clip(W[:, n0 + j, :] * FP8_WSCALE, -240, 240).astype(f8)
                w = w.reshape(T2, 128, 2, HD)          # [t2, p, i, d]
                out[j] = w.transpose(1, 0, 2, 3).reshape(128, T2 * 2 * HD)
            return out

        inp["wq"] = w_slices8(Wq)
        inp["wk"] = w_slices8(Wk)
        bscale = FP8_HSCALE * FP8_WSCALE
    else:
        def w_slices(W):
            out = np.empty((NHC_, 128, T * HD), f16)
            for j in range(NHC_):
                w = W[:, n0 + j, :].reshape(T, 128, HD)     # [t, p, d]
                out[j] = w.transpose(1, 0, 2).reshape(128, T * HD)
            return out

        inp["wq"] = w_slices(Wq)
        inp["wk"] = w_slices(Wk)
        bscale = 1.0

    # [t, p, x] -> [p, (t x)] so the device DMA is contiguous
    wvt = np.ascontiguousarray(
        Wv[:, n0:n0 + NHC_, :]).reshape(T, 128, NHC_ * HD).astype(f16)
    inp["wvT"] = np.ascontiguousarray(wvt.transpose(1, 0, 2)).reshape(
        128, T * NHC_ * HD)
    # [j, p, h] -> [p, (j h)]
    wot = np.ascontiguousarray(Wo[n0:n0 + NHC_]).astype(f16)
    inp["woT"] = np.ascontiguousarray(wot.transpose(1, 0, 2)).reshape(
        128, NHC_ * H_)

    inp["bqT"] = np.ascontiguousarray(bq[n0:n0 + NHC_].T * bscale).astype(np.float32)
    inp["bkT"] = np.ascontiguousarray(bk[n0:n0 + NHC_].T * bscale).astype(np.float32)
    inp["bv4"] = bv[n0:n0 + NHC_].reshape(1, NHC_ * HD).astype(f16)

    swap = np.zeros((128, 128), f16)
    m_idx = np.arange(128)
    swap[(m_idx + 64) % 128, m_idx] = 1.0
    inp["swapT"] = swap

    # causal mask checks + shared diagonal block [128, i*512 + q], i = kt-4c
    m01 = (mask_b <= 0.5).astype(np.float32).T      # [k, q] keep-mask
    KT, CC = S_ // 128, S_ // 512
    m4 = m01.reshape(KT, 128, CC, 512)              # [kt, p, c, q]
    mdiag = None
    for c in range(CC):
        blk = m4[4 * c:4 * c + 4, :, c, :]          # [4, 128, 512]
        if mdiag is None:
            mdiag = blk
        else:
            assert np.array_equal(blk, mdiag), "mask diagonal blocks differ"
        assert m4[: 4 * c, :, c, :].all(), "mask below diagonal not all-keep"
        assert not m4[4 * c + 4:, :, c, :].any(), "mask above diagonal not all-drop"
    inp["mdiag"] = np.ascontiguousarray(
        mdiag.transpose(1, 0, 2)).reshape(128, 4 * 512).astype(f16)
    return inp


def kernel(hidden_states, mask, Wq, bq, Wk, bk, Wv, bv, Wo, bo):
    global LAST_RESULTS
    from concourse.bass_utils import run_bass_kernel_spmd

    hidden_states = np.asarray(hidden_states, dtype=np.float32)
    mask = np.asarray(mask, dtype=np.float32)
    Wq, bq = np.asarray(Wq, np.float32), np.asarray(bq, np.float32)
    Wk, bk = np.asarray(Wk, np.float32), np.asarray(bk, np.float32)
    Wv, bv = np.asarray(Wv, np.float32), np.asarray(bv, np.float32)
    Wo, bo = np.asarray(Wo, np.float32), np.asarray(bo, np.float32)

    cosT, ssT = _rope_tables(S)
    in_maps = []
    for core in range(N_CORES):
        b = core // HGRID
        n0 = (core % HGRID) * NHC
        in_maps.append(prep_core_inputs(
            hidden_states[b], mask[b, 0], Wq, bq, Wk, bk, Wv, bv, Wo,
            n0, S, H, NHC, cosT, ssT, USE_FP8))

    key = (S, H, NHC, USE_FP8)
    if key not in _CACHE:
        _CACHE[key] = build_program(S, H, NHC, USE_FP8)
    nc = _CACHE[key]

    res = run_bass_kernel_spmd(nc, in_maps, core_ids=list(range(N_CORES)))
    LAST_RESULTS = res

    out = np.zeros((B, S, H), np.float32)
    for core in range(N_CORES):
        out[core // HGRID] += res.results[core]["o"].astype(np.float32)
    out += bo[None, None, :]
    return out
